# revision 18
# baseline (speedup 1.0000x reference)
"""Trainium2 Bass kernel for nn_C3SNN_ModelT: CNN feature extractor + LIF SNN.

Data parallel over 8 cores (128 samples each). Per core:
  - conv stage: 3x (conv3x3 SAME + relu + maxpool2x2), fp32 matmuls.
    L1 uses a DRAM-staged im2col (K=27, single pass); L2/L3 use ky-replicated
    padded rows with kx handled by accumulating matmul passes. Col-tiled PSUM
    packing keeps relu/pool epilogues on all 128 partitions; pooling runs
    before relu (they commute) straight out of PSUM via reduce_max.
  - SNN stage: 32 timesteps, feature-major layout (features on partitions,
    batch in free dim). FC matmuls use fp16 split weights (w = hi + lo, both
    fp16); spike inputs are {0,1} hence exact in fp16; PSUM accumulates fp32.
    LIF state updates are fused scalar_tensor_tensor ops on DVE; spike
    extraction runs on GPSIMD.
"""
import sys
sys.path.insert(0, "/opt/trn_rl_repo")

import numpy as np
import concourse.bass as bass
import concourse.mybir as mybir
import concourse.tile as tile
from concourse import bacc
from concourse.bass_utils import run_bass_kernel_spmd

F32 = mybir.dt.float32
F16 = mybir.dt.float16
MAX = mybir.AluOpType.max
MULT = mybir.AluOpType.mult
ADD = mybir.AluOpType.add
IS_GT = mybir.AluOpType.is_gt
IS_LE = mybir.AluOpType.is_le
RELU = mybir.ActivationFunctionType.Relu
SIGN = mybir.ActivationFunctionType.Sign
AXX = mybir.AxisListType.X

N_CORES = 8
BPC = 128          # batch per core
BB = 8             # conv batch chunk
NCHUNK = BPC // BB
SEQ = 32

LAST_EXEC_NS = None
_CACHE = {}


def build_nc(debug_outputs=False, do_conv=True, seq=SEQ):
    nc = bacc.Bacc(None, target_bir_lowering=False, debug=False)

    # ---- DRAM I/O ----
    xp = nc.dram_tensor("xp", [BPC, 3, 34, 34], F32, kind="ExternalInput")
    w1g = nc.dram_tensor("w1g", [27, 32], F32, kind="ExternalInput")
    im27d = nc.dram_tensor("im27d", [27, BPC, 1088], F32)
    w2g = nc.dram_tensor("w2g", [3, 96, 64], F32, kind="ExternalInput")
    w3a = nc.dram_tensor("w3a", [3, 128, 64], F32, kind="ExternalInput")
    w3b = nc.dram_tensor("w3b", [3, 64, 64], F32, kind="ExternalInput")
    cb1 = nc.dram_tensor("cb1", [128, 1], F32, kind="ExternalInput")
    cb2 = nc.dram_tensor("cb2", [128, 1], F32, kind="ExternalInput")
    cb3 = nc.dram_tensor("cb3", [128, 1], F32, kind="ExternalInput")  # 0.4*b3
    fc1h = nc.dram_tensor("fc1h", [128, 8 * 4 * 128], F16, kind="ExternalInput")
    fc1l = nc.dram_tensor("fc1l", [128, 8 * 4 * 128], F16, kind="ExternalInput")
    fc2h = nc.dram_tensor("fc2h", [128, 4 * 2 * 128], F16, kind="ExternalInput")
    fc2l = nc.dram_tensor("fc2l", [128, 4 * 2 * 128], F16, kind="ExternalInput")
    lih = nc.dram_tensor("lih", [128, 2 * 10], F16, kind="ExternalInput")
    lil = nc.dram_tensor("lil", [128, 2 * 10], F16, kind="ExternalInput")
    id10 = nc.dram_tensor("id10", [10, 10], F32, kind="ExternalInput")
    out = nc.dram_tensor("out", [BPC, 10], F32, kind="ExternalOutput")
    dbg = {}
    if debug_outputs:
        dbg["featT"] = nc.dram_tensor("dbg_featT", [128, 8, 128], F32,
                                      kind="ExternalOutput")

    xr = xp[:].rearrange("b c h w -> c b (h w)")

    with tile.TileContext(nc) as tc:
        with (
            tc.tile_pool(name="wpool", bufs=1) as wpool,
            tc.tile_pool(name="state", bufs=1) as state,
        ):
            # weights to SBUF
            w1s = wpool.tile([27, 32], F32)
            w2s = wpool.tile([96, 3, 64], F32)
            w3as = wpool.tile([128, 3, 64], F32)
            w3bs = wpool.tile([64, 3, 64], F32)
            cb1s = wpool.tile([128, 1], F32)
            cb2s = wpool.tile([128, 1], F32)
            cb3s = wpool.tile([128, 1], F32)
            fc1hs = wpool.tile([128, 8 * 4 * 128], F16)
            fc1ls = wpool.tile([128, 8 * 4 * 128], F16)
            fc2hs = wpool.tile([128, 4 * 2 * 128], F16)
            fc2ls = wpool.tile([128, 4 * 2 * 128], F16)
            lihs = wpool.tile([128, 2 * 10], F16)
            lils = wpool.tile([128, 2 * 10], F16)
            id10s = wpool.tile([10, 10], F32)
            for dst_t, src_t in [(w1s, w1g), (cb1s, cb1), (cb2s, cb2),
                                 (cb3s, cb3), (fc1hs, fc1h), (fc1ls, fc1l),
                                 (fc2hs, fc2h), (fc2ls, fc2l), (lihs, lih),
                                 (lils, lil), (id10s, id10)]:
                nc.sync.dma_start(dst_t[:], src_t[:])
            for dst_t, src_t in [(w2s, w2g), (w3as, w3a), (w3bs, w3b)]:
                nc.sync.dma_start(dst_t[:],
                                  src_t[:].rearrange("k p n -> p k n"))

            # featT: scaled features (0.1*feat), f-layout [p=(sig,ch), t(8), b]
            featT = state.tile([128, 8, 128], F32)

            if do_conv:
                build_conv(nc, tc, xr, im27d, featT, w1s, w2s, w3as, w3bs,
                           cb1s, cb2s, cb3s)
            else:
                nc.vector.memset(featT[:], 0.0)

            if debug_outputs:
                nc.sync.dma_start(dbg["featT"][:], featT[:])

            build_snn(nc, tc, state, featT, fc1hs, fc1ls, fc2hs, fc2ls,
                      lihs, lils, id10s, out, seq)

    nc.compile()
    return nc


def build_conv(nc, tc, xr, im27d, featT, w1s, w2s, w3as, w3bs,
               cb1s, cb2s, cb3s):
    # L1 im2col staged in DRAM: row p=(kx,ky,ci) holds padded rows shifted by
    # (ky, kx): im27d[p, b, i*34+j'] = xpad[ci, b, i+ky, j'+kx] via contiguous
    # runs; run-wrap garbage lands only in never-read pad columns j' >= 32.
    # Staged per chunk inside the loop so the DRAM->DRAM copies pipeline with
    # compute; HWDGE FIFO order on the sync queue sequences D2D before the
    # chunk's D2S load.
    im27v = im27d[:]

    with (
        tc.tile_pool(name="conv_in", bufs=1) as conv_in,
        tc.tile_pool(name="conv_sc", bufs=2) as csc,
        tc.tile_pool(name="pl1", bufs=2, space="PSUM") as pl1,
        tc.tile_pool(name="pl23", bufs=2, space="PSUM") as pl23,
    ):
        # layout tiles; padded borders memset once: per-chunk DMAs only write
        # real interiors, the boundary zeros persist across chunks
        t27s = [conv_in.tile([27, BB, 1088], F32, tag=f"t27_{i}",
                             name=f"t27_{i}") for i in range(2)]
        l2pad = conv_in.tile([32, BB, 18, 18], F32, tag="l2p", name="l2p")
        rep96 = conv_in.tile([96, BB, 16, 18], F32, tag="r96", name="r96")
        l3pad = conv_in.tile([64, BB, 10, 10], F32, tag="l3p", name="l3p")
        repa = conv_in.tile([128, BB, 8, 10], F32, tag="ra", name="ra")
        repb = conv_in.tile([64, BB, 8, 10], F32, tag="rb", name="rb")
        nc.vector.memset(l2pad[:], 0.0)
        nc.vector.memset(l3pad[:], 0.0)

        for ci in range(NCHUNK):
            b0 = ci * BB
            t27 = t27s[ci % 2]
            for kx in range(3):
                for ky in range(3):
                    p0 = 3 * (kx * 3 + ky)
                    s0 = ky * 34 + kx
                    L = min(1088, 1156 - s0)
                    nc.sync.dma_start(im27v[p0:p0 + 3, b0:b0 + BB, 0:L],
                                      xr[0:3, b0:b0 + BB, s0:s0 + L])
                    if L < 1088:
                        # junk tail lands in never-read pad cols; keeps
                        # CoreSim's uninitialized-read checks quiet
                        with nc.allow_non_contiguous_dma(reason="pad tail"):
                            nc.sync.dma_start(
                                im27v[p0:p0 + 3, b0:b0 + BB, L:1088],
                                xr[0:3, b0:b0 + BB, 0:1088 - L])
            # ---- L1: load staged im2col chunk; 4 rounds x 4 col-groups ----
            nc.sync.dma_start(t27[:], im27v[0:27, b0:b0 + BB, :])
            t27view = t27[:].rearrange("p b (i j) -> p b i j", j=34)
            for rnd in range(4):
                ps = pl1.tile([128, 512], F32, tag="ps1", name="ps1")
                for c in range(4):
                    u = rnd * 4 + c
                    smp, nh = u // 2, u % 2
                    nc.tensor.matmul(
                        ps[32 * c:32 * c + 32, :], w1s[:, :],
                        t27view[0:27, smp, 16 * nh:16 * nh + 16, 0:32],
                        start=True, stop=True, tile_position=(0, 32 * c))
                r4 = ps[:].rearrange("p (i j two) -> p i j two",
                                     i=16, j=16, two=2)
                p1t = csc.tile([128, 16, 16], F32, tag="cpa", name="cpa1")
                nc.vector.reduce_max(p1t[:], r4, axis=AXX)
                p14 = p1t[:].rearrange("p (i two) j -> p i two j", two=2)
                p2t = csc.tile([128, 8, 16], F32, tag="cpb", name="cpb1")
                nc.vector.tensor_tensor(p2t[:], p14[:, :, 0, :],
                                        p14[:, :, 1, :], MAX)
                p2r = csc.tile([128, 8, 16], F32, tag="cpr", name="cpr1")
                nc.scalar.activation(p2r[:], p2t[:], RELU, bias=cb1s[:])
                for c in range(4):
                    u = rnd * 4 + c
                    smp, nh = u // 2, u % 2
                    nc.sync.dma_start(
                        l2pad[0:32, smp, 1 + 8 * nh:9 + 8 * nh, 1:17],
                        p2r[32 * c:32 * c + 32, :, :])

            # ---- L2: ky-replicate + 3 kx passes, col-pack x2 ----
            for ky in range(3):
                nc.sync.dma_start(rep96[32 * ky:32 * ky + 32, :],
                                  l2pad[0:32, :, ky:ky + 16, :])
            for n2 in range(2):
                ps = pl23.tile([128, 512], F32, tag="ps2", name="ps2")
                for c in range(2):
                    for kx in range(3):
                        nc.tensor.matmul(
                            ps[64 * c:64 * c + 64, :], w2s[:, kx, :],
                            rep96[0:96, c * 4 + n2 * 2:c * 4 + n2 * 2 + 2,
                                  :, kx:kx + 16],
                            start=(kx == 0), stop=(kx == 2),
                            tile_position=(0, 64 * c))
                r4 = ps[:].rearrange("p (si j two) -> p si j two",
                                     si=32, j=8, two=2)
                p1t = csc.tile([128, 32, 8], F32, tag="cpa", name="cpa2")
                nc.vector.reduce_max(p1t[:], r4, axis=AXX)
                p14 = p1t[:].rearrange("p (a two) j -> p a two j", two=2)
                p2t = csc.tile([128, 2, 8, 8], F32, tag="cpb", name="cpb2")
                p2tv = p2t[:].rearrange("p s i j -> p (s i) j")
                nc.vector.tensor_tensor(p2tv, p14[:, :, 0, :],
                                        p14[:, :, 1, :], MAX)
                p2r = csc.tile([128, 2, 8, 8], F32, tag="cpr", name="cpr2")
                nc.scalar.activation(p2r[:], p2t[:], RELU, bias=cb2s[:])
                for c in range(2):
                    s0 = c * 4 + n2 * 2
                    for si in range(2):
                        nc.sync.dma_start(
                            l3pad[0:64, s0 + si, 1:9, 1:9],
                            p2r[64 * c:64 * c + 64, si, :, :])

            # ---- L3: ky-replicate + matmuls, col-pack x2 ----
            for ky in range(2):
                nc.sync.dma_start(repa[64 * ky:64 * ky + 64, :],
                                  l3pad[0:64, :, ky:ky + 8, :])
            nc.sync.dma_start(repb[0:64, :], l3pad[0:64, :, 2:10, :])
            ps3 = pl23.tile([128, 256], F32, tag="ps3", name="ps3")
            for c in range(2):
                for kx in range(3):
                    nc.tensor.matmul(
                        ps3[64 * c:64 * c + 64, :], w3as[:, kx, :],
                        repa[0:128, c * 4:c * 4 + 4, :, kx:kx + 8],
                        start=(kx == 0), stop=False,
                        tile_position=(0, 64 * c))
                    nc.tensor.matmul(
                        ps3[64 * c:64 * c + 64, :], w3bs[:, kx, :],
                        repb[0:64, c * 4:c * 4 + 4, :, kx:kx + 8],
                        start=False, stop=(kx == 2),
                        tile_position=(0, 64 * c))
            r4 = ps3[:].rearrange("p (si j two) -> p si j two",
                                  si=32, j=4, two=2)
            p1t = csc.tile([128, 32, 4], F32, tag="cpa", name="cpa3")
            nc.vector.reduce_max(p1t[:], r4, axis=AXX)
            p14 = p1t[:].rearrange("p (s i two) j -> p s i two j",
                                   s=4, i=4, two=2)
            # pass2 writes (q, s)-major flat layout: elem q*4 + s
            p2p = csc.tile([128, 64], F32, tag="cpb", name="cpb3")
            p2pv = p2p[:].rearrange("p (i j s) -> p s i j", i=4, j=4, s=4)
            nc.vector.tensor_tensor(p2pv, p14[:, :, :, 0, :],
                                    p14[:, :, :, 1, :], MAX)
            # relu(0.4*x + 0.4*b3) = 0.4*relu(x + b3); folds CNN_SCALER*DT_TM
            p2t = csc.tile([128, 64], F32, tag="cpr", name="cpr3")
            nc.scalar.activation(p2t[:], p2p[:], RELU, bias=cb3s[:], scale=0.4)
            # featT assembly: spatial q = i*4+j = 2t + sig; feature f = q*64+ch
            p2q = p2t[:].rearrange("p (t two s) -> p t two s", t=8, two=2, s=4)
            for sig in range(2):
                for c in range(2):
                    src = p2q[64 * c:64 * c + 64, :, sig, :]
                    dst = featT[64 * sig:64 * sig + 64, :,
                                b0 + 4 * c:b0 + 4 * c + 4]
                    if sig == c:
                        nc.vector.tensor_copy(dst.opt(), src.opt())
                    else:
                        nc.sync.dma_start(dst.opt(), src.opt())


def build_snn(nc, tc, state, featT, fc1hs, fc1ls, fc2hs, fc2ls, lihs, lils,
              id10s, out, seq):
    with (
        tc.tile_pool(name="snn_sc", bufs=1) as ssc,
        tc.tile_pool(name="pc1", bufs=2, space="PSUM") as pc1,
        tc.tile_pool(name="pc2", bufs=2, space="PSUM") as pc2,
        tc.tile_pool(name="pli", bufs=2, space="PSUM") as pli,
    ):
        ve = state.tile([128, 8, 128], F32)
        vs1 = state.tile([128, 4, 128], F32)   # 10*v1
        i1 = state.tile([128, 4, 128], F32)
        vs2 = state.tile([128, 2, 128], F32)   # 10*v2
        i2 = state.tile([128, 2, 128], F32)
        wl = state.tile([10, 128], F32)        # 10*vl
        il = state.tile([10, 128], F32)
        z16 = state.tile([128, 8, 128], F16)
        s116 = state.tile([128, 4, 128], F16)
        s216 = state.tile([128, 2, 128], F16)
        for t_ in (ve, vs1, i1, vs2, i2, wl, il):
            nc.vector.memset(t_[:], 0.0)

        fc1h4 = fc1hs.rearrange("p (k m n) -> p k m n", k=8, m=4)
        fc1l4 = fc1ls.rearrange("p (k m n) -> p k m n", k=8, m=4)
        fc2h4 = fc2hs.rearrange("p (k m n) -> p k m n", k=4, m=2)
        fc2l4 = fc2ls.rearrange("p (k m n) -> p k m n", k=4, m=2)
        lih4 = lihs.rearrange("p (k n) -> p k n", k=2)
        lil4 = lils.rearrange("p (k n) -> p k n", k=2)

        for t in range(seq):
            # encoder: ve = 0.9*ve + 0.1*feat; z = ve>1; ve *= (ve<=1)
            nc.vector.scalar_tensor_tensor(
                ve[:], ve[:], 0.9, featT[:], MULT, ADD)
            nc.gpsimd.tensor_scalar(z16[:], ve[:], 1.0, None, IS_GT)
            nc.vector.scalar_tensor_tensor(
                ve[:], ve[:], 1.0, ve[:], IS_LE, MULT)

            # fc1: cur1 = fc1_w @ z  (f-layout out [512, 128])
            ps1 = pc1.tile([128, 4, 128], F32, tag="ps1", name="sps1")
            for m in range(4):
                for k in range(8):
                    nc.tensor.matmul(
                        ps1[:, m, :], fc1h4[:, k, m, :], z16[:, k, :],
                        start=(k == 0), stop=False)
                for k in range(8):
                    nc.tensor.matmul(
                        ps1[:, m, :], fc1l4[:, k, m, :], z16[:, k, :],
                        start=False, stop=(k == 7))

            # LIF1 (state scaled by 10; th=4.0): v1d uses OLD i1
            v1d = ssc.tile([128, 4, 128], F32, tag="scrA", name="v1d")
            nc.vector.scalar_tensor_tensor(
                v1d[:], vs1[:], 0.9, i1[:], MULT, ADD)
            nc.gpsimd.tensor_scalar(s116[:], v1d[:], 4.0, None, IS_GT)
            nc.vector.scalar_tensor_tensor(
                vs1[:], v1d[:], 4.0, v1d[:], IS_LE, MULT)
            nc.vector.scalar_tensor_tensor(
                i1[:], i1[:], 0.8, ps1[:], MULT, ADD)

            # fc2
            ps2 = pc2.tile([128, 2, 128], F32, tag="ps2", name="sps2")
            for m in range(2):
                for k in range(4):
                    nc.tensor.matmul(
                        ps2[:, m, :], fc2h4[:, k, m, :], s116[:, k, :],
                        start=(k == 0), stop=False)
                for k in range(4):
                    nc.tensor.matmul(
                        ps2[:, m, :], fc2l4[:, k, m, :], s116[:, k, :],
                        start=False, stop=(k == 3))

            # LIF2
            v2d = ssc.tile([128, 2, 128], F32, tag="scrA", name="v2d")
            nc.vector.scalar_tensor_tensor(
                v2d[:], vs2[:], 0.9, i2[:], MULT, ADD)
            nc.gpsimd.tensor_scalar(s216[:], v2d[:], 4.0, None, IS_GT)
            nc.vector.scalar_tensor_tensor(
                vs2[:], v2d[:], 4.0, v2d[:], IS_LE, MULT)
            nc.vector.scalar_tensor_tensor(
                i2[:], i2[:], 0.8, ps2[:], MULT, ADD)

            # LILinear: ij = il + li_w @ s2; wl = 0.9wl + ij; il = 0.8*ij
            psl = pli.tile([10, 128], F32, tag="psl", name="psl")
            nc.tensor.matmul(psl[:], lih4[:, 0, :], s216[:, 0, :],
                             start=True, stop=False)
            nc.tensor.matmul(psl[:], lih4[:, 1, :], s216[:, 1, :],
                             start=False, stop=False)
            nc.tensor.matmul(psl[:], lil4[:, 0, :], s216[:, 0, :],
                             start=False, stop=False)
            nc.tensor.matmul(psl[:], lil4[:, 1, :], s216[:, 1, :],
                             start=False, stop=True)
            ij = ssc.tile([10, 128], F32, tag="scrB", name="ij")
            nc.vector.tensor_tensor(ij[:], il[:], psl[:], ADD)
            nc.vector.scalar_tensor_tensor(
                wl[:], wl[:], 0.9, ij[:], MULT, ADD)
            nc.vector.tensor_scalar(il[:], ij[:], 0.8, None, MULT)

        # output: out[b, n] = 0.1 * wl[n, b] via PE transpose
        vlT = state.tile([10, 128], F32)
        nc.vector.tensor_scalar(vlT[:], wl[:], 0.1, None, MULT)
        with tc.tile_pool(name="pout", bufs=1, space="PSUM") as pout:
            pso = pout.tile([128, 10], F32)
            nc.tensor.transpose(pso[:], vlT[:], id10s[:])
            ot = state.tile([128, 10], F32)
            nc.vector.tensor_copy(ot[:], pso[:])
            nc.sync.dma_start(out[:], ot[:])


def prep_weights(w1, b1, w2, b2, w3, b3, fc1_w, fc1_b, fc2_w, fc2_b, li_w):
    def split16(a):
        hi = a.astype(np.float16)
        lo = (a - hi.astype(np.float32)).astype(np.float16)
        return hi, lo

    d = {}
    d["w1g"] = np.ascontiguousarray(
        w1.transpose(3, 2, 1, 0).reshape(27, 32).astype(np.float32))
    d["w2g"] = np.ascontiguousarray(
        w2.transpose(3, 2, 1, 0).reshape(3, 96, 64).astype(np.float32))
    w3t = w3.transpose(3, 2, 1, 0).reshape(3, 192, 64).astype(np.float32)
    d["w3a"] = np.ascontiguousarray(w3t[:, :128])
    d["w3b"] = np.ascontiguousarray(w3t[:, 128:])
    d["cb1"] = np.tile(b1.astype(np.float32), 4).reshape(128, 1)
    d["cb2"] = np.tile(b2.astype(np.float32), 2).reshape(128, 1)
    d["cb3"] = (0.4 * np.tile(b3.astype(np.float32), 2)).reshape(128, 1)
    # fc1: permute input features to f=(s, c) ordering; tiles [p, k, m, n]
    perm = np.array([c * 16 + s for s in range(16) for c in range(64)])
    fc1t = fc1_w.T[perm].astype(np.float32)            # [1024, 512]
    a = fc1t.reshape(8, 128, 4, 128).transpose(1, 0, 2, 3).reshape(128, -1)
    d["fc1h"], d["fc1l"] = split16(a)
    fc2t = fc2_w.T.astype(np.float32)                  # [512, 256]
    a = fc2t.reshape(4, 128, 2, 128).transpose(1, 0, 2, 3).reshape(128, -1)
    d["fc2h"], d["fc2l"] = split16(a)
    lit = li_w.T.astype(np.float32)                    # [256, 10]
    a = lit.reshape(2, 128, 10).transpose(1, 0, 2).reshape(128, 20)
    d["lih"], d["lil"] = split16(a)
    d["id10"] = np.eye(10, dtype=np.float32)
    assert not np.any(fc1_b) and not np.any(fc2_b), \
        "nonzero fc biases not implemented"
    return d


def kernel(x, w1, b1, w2, b2, w3, b3, fc1_w, fc1_b, fc2_w, fc2_b, li_w,
           trace=False):
    global LAST_EXEC_NS
    if "nc" not in _CACHE:
        _CACHE["nc"] = build_nc()
    nc = _CACHE["nc"]
    wd = prep_weights(w1, b1, w2, b2, w3, b3, fc1_w, fc1_b, fc2_w, fc2_b, li_w)
    in_maps = []
    for c in range(N_CORES):
        m = dict(wd)
        xs = x[c * BPC:(c + 1) * BPC].astype(np.float32)
        m["xp"] = np.pad(xs, ((0, 0), (0, 0), (1, 1), (1, 1)))
        in_maps.append(m)
    res = run_bass_kernel_spmd(nc, in_maps, list(range(N_CORES)), trace=trace)
    LAST_EXEC_NS = res.exec_time_ns
    return np.concatenate([res.results[c]["out"] for c in range(N_CORES)], 0)


# revision 19
# speedup vs baseline: 1.0436x; 1.0436x over previous
"""Trainium2 Bass kernel for nn_C3SNN_ModelT: CNN feature extractor + LIF SNN.

Data parallel over 8 cores (128 samples each). Per core:
  - conv stage: 3x (conv3x3 SAME + relu + maxpool2x2), fp32 matmuls.
    L1 uses a DRAM-staged im2col (K=27, single pass); L2/L3 use ky-replicated
    padded rows with kx handled by accumulating matmul passes. Col-tiled PSUM
    packing keeps relu/pool epilogues on all 128 partitions; pooling runs
    before relu (they commute) straight out of PSUM via reduce_max.
  - SNN stage: 32 timesteps, feature-major layout (features on partitions,
    batch in free dim). FC matmuls use fp16 split weights (w = hi + lo, both
    fp16); spike inputs are {0,1} hence exact in fp16; PSUM accumulates fp32.
    LIF state updates are fused scalar_tensor_tensor ops on DVE; spike
    extraction runs on GPSIMD.
"""
import sys
sys.path.insert(0, "/opt/trn_rl_repo")

import numpy as np
import concourse.bass as bass
import concourse.mybir as mybir
import concourse.tile as tile
from concourse import bacc
from concourse.bass_utils import run_bass_kernel_spmd

F32 = mybir.dt.float32
F16 = mybir.dt.float16
MAX = mybir.AluOpType.max
MULT = mybir.AluOpType.mult
ADD = mybir.AluOpType.add
IS_GT = mybir.AluOpType.is_gt
IS_LE = mybir.AluOpType.is_le
RELU = mybir.ActivationFunctionType.Relu
SIGN = mybir.ActivationFunctionType.Sign
AXX = mybir.AxisListType.X

N_CORES = 8
BPC = 128          # batch per core
BB = 8             # conv batch chunk
NCHUNK = BPC // BB
SEQ = 32

LAST_EXEC_NS = None
_CACHE = {}


def build_nc(debug_outputs=False, do_conv=True, seq=SEQ):
    nc = bacc.Bacc(None, target_bir_lowering=False, debug=False)

    # ---- DRAM I/O ----
    xp = nc.dram_tensor("xp", [BPC, 3, 34, 34], F32, kind="ExternalInput")
    w1g = nc.dram_tensor("w1g", [27, 32], F32, kind="ExternalInput")
    w2g = nc.dram_tensor("w2g", [3, 96, 64], F32, kind="ExternalInput")
    w3a = nc.dram_tensor("w3a", [3, 128, 64], F32, kind="ExternalInput")
    w3b = nc.dram_tensor("w3b", [3, 64, 64], F32, kind="ExternalInput")
    cb1 = nc.dram_tensor("cb1", [128, 1], F32, kind="ExternalInput")
    cb2 = nc.dram_tensor("cb2", [128, 1], F32, kind="ExternalInput")
    cb3 = nc.dram_tensor("cb3", [128, 1], F32, kind="ExternalInput")  # 0.4*b3
    fc1h = nc.dram_tensor("fc1h", [128, 8 * 4 * 128], F16, kind="ExternalInput")
    fc1l = nc.dram_tensor("fc1l", [128, 8 * 4 * 128], F16, kind="ExternalInput")
    fc2h = nc.dram_tensor("fc2h", [128, 4 * 2 * 128], F16, kind="ExternalInput")
    fc2l = nc.dram_tensor("fc2l", [128, 4 * 2 * 128], F16, kind="ExternalInput")
    lih = nc.dram_tensor("lih", [128, 2 * 10], F16, kind="ExternalInput")
    lil = nc.dram_tensor("lil", [128, 2 * 10], F16, kind="ExternalInput")
    id10 = nc.dram_tensor("id10", [10, 10], F32, kind="ExternalInput")
    out = nc.dram_tensor("out", [BPC, 10], F32, kind="ExternalOutput")
    dbg = {}
    if debug_outputs:
        dbg["featT"] = nc.dram_tensor("dbg_featT", [128, 8, 128], F32,
                                      kind="ExternalOutput")

    xr = xp[:].rearrange("b c h w -> c b (h w)")

    with tile.TileContext(nc) as tc:
        with (
            tc.tile_pool(name="wpool", bufs=1) as wpool,
            tc.tile_pool(name="state", bufs=1) as state,
        ):
            # weights to SBUF
            w1s = wpool.tile([27, 32], F32)
            w2s = wpool.tile([96, 3, 64], F32)
            w3as = wpool.tile([128, 3, 64], F32)
            w3bs = wpool.tile([64, 3, 64], F32)
            cb1s = wpool.tile([128, 1], F32)
            cb2s = wpool.tile([128, 1], F32)
            cb3s = wpool.tile([128, 1], F32)
            fc1hs = wpool.tile([128, 8 * 4 * 128], F16)
            fc1ls = wpool.tile([128, 8 * 4 * 128], F16)
            fc2hs = wpool.tile([128, 4 * 2 * 128], F16)
            fc2ls = wpool.tile([128, 4 * 2 * 128], F16)
            lihs = wpool.tile([128, 2 * 10], F16)
            lils = wpool.tile([128, 2 * 10], F16)
            id10s = wpool.tile([10, 10], F32)
            for dst_t, src_t in [(w1s, w1g), (cb1s, cb1), (cb2s, cb2),
                                 (cb3s, cb3), (fc1hs, fc1h), (fc1ls, fc1l),
                                 (fc2hs, fc2h), (fc2ls, fc2l), (lihs, lih),
                                 (lils, lil), (id10s, id10)]:
                nc.sync.dma_start(dst_t[:], src_t[:])
            for dst_t, src_t in [(w2s, w2g), (w3as, w3a), (w3bs, w3b)]:
                nc.sync.dma_start(dst_t[:],
                                  src_t[:].rearrange("k p n -> p k n"))

            # featT: scaled features (0.1*feat), f-layout [p=(sig,ch), t(8), b]
            featT = state.tile([128, 8, 128], F32)

            if do_conv:
                build_conv(nc, tc, xr, featT, w1s, w2s, w3as, w3bs,
                           cb1s, cb2s, cb3s)
            else:
                nc.vector.memset(featT[:], 0.0)

            if debug_outputs:
                nc.sync.dma_start(dbg["featT"][:], featT[:])

            build_snn(nc, tc, state, featT, fc1hs, fc1ls, fc2hs, fc2ls,
                      lihs, lils, id10s, out, seq)

    nc.compile()
    return nc


def build_conv(nc, tc, xr, featT, w1s, w2s, w3as, w3bs,
               cb1s, cb2s, cb3s):
    # L1 im2col staged in DRAM: row p=(kx,ky,ci) holds padded rows shifted by
    # (ky, kx): im27d[p, b, i*34+j'] = xpad[ci, b, i+ky, j'+kx] via contiguous
    # runs; run-wrap garbage lands only in never-read pad columns j' >= 32.
    # Staged per chunk inside the loop so the DRAM->DRAM copies pipeline with
    # compute; HWDGE FIFO order on the sync queue sequences D2D before the
    # chunk's D2S load.
    with (
        tc.tile_pool(name="conv_in", bufs=1) as conv_in,
        tc.tile_pool(name="conv_sc", bufs=2) as csc,
        tc.tile_pool(name="dstage", bufs=1, space="DRAM") as dstage,
        tc.tile_pool(name="pl1", bufs=2, space="PSUM") as pl1,
        tc.tile_pool(name="pl23", bufs=2, space="PSUM") as pl23,
    ):
        im27t = dstage.tile([27, BPC, 1088], F32)
        im27v = im27t[:]
        # layout tiles; padded borders memset once: per-chunk DMAs only write
        # real interiors, the boundary zeros persist across chunks
        t27s = [conv_in.tile([27, BB, 1088], F32, tag=f"t27_{i}",
                             name=f"t27_{i}") for i in range(2)]
        l2pad = conv_in.tile([32, BB, 18, 18], F32, tag="l2p", name="l2p")
        rep96 = conv_in.tile([96, BB, 16, 18], F32, tag="r96", name="r96")
        l3pad = conv_in.tile([64, BB, 10, 10], F32, tag="l3p", name="l3p")
        repa = conv_in.tile([128, BB, 8, 10], F32, tag="ra", name="ra")
        repb = conv_in.tile([64, BB, 8, 10], F32, tag="rb", name="rb")
        nc.vector.memset(l2pad[:], 0.0)
        nc.vector.memset(l3pad[:], 0.0)

        for ci in range(NCHUNK):
            b0 = ci * BB
            t27 = t27s[ci % 2]
            for kx in range(3):
                for ky in range(3):
                    p0 = 3 * (kx * 3 + ky)
                    s0 = ky * 34 + kx
                    L = min(1088, 1156 - s0)
                    nc.gpsimd.dma_start(im27v[p0:p0 + 3, b0:b0 + BB, 0:L],
                                        xr[0:3, b0:b0 + BB, s0:s0 + L])
                    if L < 1088:
                        # junk tail lands in never-read pad cols; keeps
                        # CoreSim's uninitialized-read checks quiet
                        with nc.allow_non_contiguous_dma(reason="pad tail"):
                            nc.gpsimd.dma_start(
                                im27v[p0:p0 + 3, b0:b0 + BB, L:1088],
                                xr[0:3, b0:b0 + BB, 0:1088 - L])
            # ---- L1: load staged im2col chunk; 4 rounds x 4 col-groups ----
            nc.sync.dma_start(t27[:], im27v[0:27, b0:b0 + BB, :])
            t27view = t27[:].rearrange("p b (i j) -> p b i j", j=34)
            for rnd in range(4):
                ps = pl1.tile([128, 512], F32, tag="ps1", name="ps1")
                for c in range(4):
                    u = rnd * 4 + c
                    smp, nh = u // 2, u % 2
                    nc.tensor.matmul(
                        ps[32 * c:32 * c + 32, :], w1s[:, :],
                        t27view[0:27, smp, 16 * nh:16 * nh + 16, 0:32],
                        start=True, stop=True, tile_position=(0, 32 * c))
                r4 = ps[:].rearrange("p (i j two) -> p i j two",
                                     i=16, j=16, two=2)
                p1t = csc.tile([128, 16, 16], F32, tag="cpa", name="cpa1")
                nc.vector.reduce_max(p1t[:], r4, axis=AXX)
                p14 = p1t[:].rearrange("p (i two) j -> p i two j", two=2)
                p2t = csc.tile([128, 8, 16], F32, tag="cpb", name="cpb1")
                nc.vector.tensor_tensor(p2t[:], p14[:, :, 0, :],
                                        p14[:, :, 1, :], MAX)
                p2r = csc.tile([128, 8, 16], F32, tag="cpr", name="cpr1")
                nc.scalar.activation(p2r[:], p2t[:], RELU, bias=cb1s[:])
                for c in range(4):
                    u = rnd * 4 + c
                    smp, nh = u // 2, u % 2
                    nc.sync.dma_start(
                        l2pad[0:32, smp, 1 + 8 * nh:9 + 8 * nh, 1:17],
                        p2r[32 * c:32 * c + 32, :, :])

            # ---- L2: ky-replicate + 3 kx passes, col-pack x2 ----
            for ky in range(3):
                nc.sync.dma_start(rep96[32 * ky:32 * ky + 32, :],
                                  l2pad[0:32, :, ky:ky + 16, :])
            for n2 in range(2):
                ps = pl23.tile([128, 512], F32, tag="ps2", name="ps2")
                for c in range(2):
                    for kx in range(3):
                        nc.tensor.matmul(
                            ps[64 * c:64 * c + 64, :], w2s[:, kx, :],
                            rep96[0:96, c * 4 + n2 * 2:c * 4 + n2 * 2 + 2,
                                  :, kx:kx + 16],
                            start=(kx == 0), stop=(kx == 2),
                            tile_position=(0, 64 * c))
                r4 = ps[:].rearrange("p (si j two) -> p si j two",
                                     si=32, j=8, two=2)
                p1t = csc.tile([128, 32, 8], F32, tag="cpa", name="cpa2")
                nc.vector.reduce_max(p1t[:], r4, axis=AXX)
                p14 = p1t[:].rearrange("p (a two) j -> p a two j", two=2)
                p2t = csc.tile([128, 2, 8, 8], F32, tag="cpb", name="cpb2")
                p2tv = p2t[:].rearrange("p s i j -> p (s i) j")
                nc.vector.tensor_tensor(p2tv, p14[:, :, 0, :],
                                        p14[:, :, 1, :], MAX)
                p2r = csc.tile([128, 2, 8, 8], F32, tag="cpr", name="cpr2")
                nc.scalar.activation(p2r[:], p2t[:], RELU, bias=cb2s[:])
                for c in range(2):
                    s0 = c * 4 + n2 * 2
                    for si in range(2):
                        nc.sync.dma_start(
                            l3pad[0:64, s0 + si, 1:9, 1:9],
                            p2r[64 * c:64 * c + 64, si, :, :])

            # ---- L3: ky-replicate + matmuls, col-pack x2 ----
            for ky in range(2):
                nc.sync.dma_start(repa[64 * ky:64 * ky + 64, :],
                                  l3pad[0:64, :, ky:ky + 8, :])
            nc.sync.dma_start(repb[0:64, :], l3pad[0:64, :, 2:10, :])
            ps3 = pl23.tile([128, 256], F32, tag="ps3", name="ps3")
            for c in range(2):
                for kx in range(3):
                    nc.tensor.matmul(
                        ps3[64 * c:64 * c + 64, :], w3as[:, kx, :],
                        repa[0:128, c * 4:c * 4 + 4, :, kx:kx + 8],
                        start=(kx == 0), stop=False,
                        tile_position=(0, 64 * c))
                    nc.tensor.matmul(
                        ps3[64 * c:64 * c + 64, :], w3bs[:, kx, :],
                        repb[0:64, c * 4:c * 4 + 4, :, kx:kx + 8],
                        start=False, stop=(kx == 2),
                        tile_position=(0, 64 * c))
            r4 = ps3[:].rearrange("p (si j two) -> p si j two",
                                  si=32, j=4, two=2)
            p1t = csc.tile([128, 32, 4], F32, tag="cpa", name="cpa3")
            nc.vector.reduce_max(p1t[:], r4, axis=AXX)
            p14 = p1t[:].rearrange("p (s i two) j -> p s i two j",
                                   s=4, i=4, two=2)
            # pass2 writes (q, s)-major flat layout: elem q*4 + s
            p2p = csc.tile([128, 64], F32, tag="cpb", name="cpb3")
            p2pv = p2p[:].rearrange("p (i j s) -> p s i j", i=4, j=4, s=4)
            nc.vector.tensor_tensor(p2pv, p14[:, :, :, 0, :],
                                    p14[:, :, :, 1, :], MAX)
            # relu(0.4*x + 0.4*b3) = 0.4*relu(x + b3); folds CNN_SCALER*DT_TM
            p2t = csc.tile([128, 64], F32, tag="cpr", name="cpr3")
            nc.scalar.activation(p2t[:], p2p[:], RELU, bias=cb3s[:], scale=0.4)
            # featT assembly: spatial q = i*4+j = 2t + sig; feature f = q*64+ch
            p2q = p2t[:].rearrange("p (t two s) -> p t two s", t=8, two=2, s=4)
            for sig in range(2):
                for c in range(2):
                    src = p2q[64 * c:64 * c + 64, :, sig, :]
                    dst = featT[64 * sig:64 * sig + 64, :,
                                b0 + 4 * c:b0 + 4 * c + 4]
                    if sig == c:
                        nc.vector.tensor_copy(dst.opt(), src.opt())
                    else:
                        nc.sync.dma_start(dst.opt(), src.opt())


def build_snn(nc, tc, state, featT, fc1hs, fc1ls, fc2hs, fc2ls, lihs, lils,
              id10s, out, seq):
    with (
        tc.tile_pool(name="snn_sc", bufs=1) as ssc,
        tc.tile_pool(name="pc1", bufs=2, space="PSUM") as pc1,
        tc.tile_pool(name="pc2", bufs=2, space="PSUM") as pc2,
        tc.tile_pool(name="pli", bufs=2, space="PSUM") as pli,
    ):
        ve = state.tile([128, 8, 128], F32)
        vs1 = state.tile([128, 4, 128], F32)   # 10*v1
        i1 = state.tile([128, 4, 128], F32)
        vs2 = state.tile([128, 2, 128], F32)   # 10*v2
        i2 = state.tile([128, 2, 128], F32)
        wl = state.tile([10, 128], F32)        # 10*vl
        il = state.tile([10, 128], F32)
        z16 = state.tile([128, 8, 128], F16)
        s116 = state.tile([128, 4, 128], F16)
        s216 = state.tile([128, 2, 128], F16)
        for t_ in (ve, vs1, i1, vs2, i2, wl, il):
            nc.vector.memset(t_[:], 0.0)

        fc1h4 = fc1hs.rearrange("p (k m n) -> p k m n", k=8, m=4)
        fc1l4 = fc1ls.rearrange("p (k m n) -> p k m n", k=8, m=4)
        fc2h4 = fc2hs.rearrange("p (k m n) -> p k m n", k=4, m=2)
        fc2l4 = fc2ls.rearrange("p (k m n) -> p k m n", k=4, m=2)
        lih4 = lihs.rearrange("p (k n) -> p k n", k=2)
        lil4 = lils.rearrange("p (k n) -> p k n", k=2)

        for t in range(seq):
            # encoder: ve = 0.9*ve + 0.1*feat; z = ve>1; ve *= (ve<=1)
            nc.vector.scalar_tensor_tensor(
                ve[:], ve[:], 0.9, featT[:], MULT, ADD)
            nc.gpsimd.tensor_scalar(z16[:], ve[:], 1.0, None, IS_GT)
            nc.vector.scalar_tensor_tensor(
                ve[:], ve[:], 1.0, ve[:], IS_LE, MULT)

            # fc1: cur1 = fc1_w @ z  (f-layout out [512, 128])
            ps1 = pc1.tile([128, 4, 128], F32, tag="ps1", name="sps1")
            for m in range(4):
                for k in range(8):
                    nc.tensor.matmul(
                        ps1[:, m, :], fc1h4[:, k, m, :], z16[:, k, :],
                        start=(k == 0), stop=False)
                for k in range(8):
                    nc.tensor.matmul(
                        ps1[:, m, :], fc1l4[:, k, m, :], z16[:, k, :],
                        start=False, stop=(k == 7))

            # LIF1 (state scaled by 10; th=4.0): v1d uses OLD i1
            v1d = ssc.tile([128, 4, 128], F32, tag="scrA", name="v1d")
            nc.vector.scalar_tensor_tensor(
                v1d[:], vs1[:], 0.9, i1[:], MULT, ADD)
            nc.gpsimd.tensor_scalar(s116[:], v1d[:], 4.0, None, IS_GT)
            nc.vector.scalar_tensor_tensor(
                vs1[:], v1d[:], 4.0, v1d[:], IS_LE, MULT)
            nc.vector.scalar_tensor_tensor(
                i1[:], i1[:], 0.8, ps1[:], MULT, ADD)

            # fc2
            ps2 = pc2.tile([128, 2, 128], F32, tag="ps2", name="sps2")
            for m in range(2):
                for k in range(4):
                    nc.tensor.matmul(
                        ps2[:, m, :], fc2h4[:, k, m, :], s116[:, k, :],
                        start=(k == 0), stop=False)
                for k in range(4):
                    nc.tensor.matmul(
                        ps2[:, m, :], fc2l4[:, k, m, :], s116[:, k, :],
                        start=False, stop=(k == 3))

            # LIF2
            v2d = ssc.tile([128, 2, 128], F32, tag="scrA", name="v2d")
            nc.vector.scalar_tensor_tensor(
                v2d[:], vs2[:], 0.9, i2[:], MULT, ADD)
            nc.gpsimd.tensor_scalar(s216[:], v2d[:], 4.0, None, IS_GT)
            nc.vector.scalar_tensor_tensor(
                vs2[:], v2d[:], 4.0, v2d[:], IS_LE, MULT)
            nc.vector.scalar_tensor_tensor(
                i2[:], i2[:], 0.8, ps2[:], MULT, ADD)

            # LILinear: ij = il + li_w @ s2; wl = 0.9wl + ij; il = 0.8*ij
            psl = pli.tile([10, 128], F32, tag="psl", name="psl")
            nc.tensor.matmul(psl[:], lih4[:, 0, :], s216[:, 0, :],
                             start=True, stop=False)
            nc.tensor.matmul(psl[:], lih4[:, 1, :], s216[:, 1, :],
                             start=False, stop=False)
            nc.tensor.matmul(psl[:], lil4[:, 0, :], s216[:, 0, :],
                             start=False, stop=False)
            nc.tensor.matmul(psl[:], lil4[:, 1, :], s216[:, 1, :],
                             start=False, stop=True)
            ij = ssc.tile([10, 128], F32, tag="scrB", name="ij")
            nc.vector.tensor_tensor(ij[:], il[:], psl[:], ADD)
            nc.vector.scalar_tensor_tensor(
                wl[:], wl[:], 0.9, ij[:], MULT, ADD)
            nc.vector.tensor_scalar(il[:], ij[:], 0.8, None, MULT)

        # output: out[b, n] = 0.1 * wl[n, b] via PE transpose
        vlT = state.tile([10, 128], F32)
        nc.vector.tensor_scalar(vlT[:], wl[:], 0.1, None, MULT)
        with tc.tile_pool(name="pout", bufs=1, space="PSUM") as pout:
            pso = pout.tile([128, 10], F32)
            nc.tensor.transpose(pso[:], vlT[:], id10s[:])
            ot = state.tile([128, 10], F32)
            nc.vector.tensor_copy(ot[:], pso[:])
            nc.sync.dma_start(out[:], ot[:])


def prep_weights(w1, b1, w2, b2, w3, b3, fc1_w, fc1_b, fc2_w, fc2_b, li_w):
    def split16(a):
        hi = a.astype(np.float16)
        lo = (a - hi.astype(np.float32)).astype(np.float16)
        return hi, lo

    d = {}
    d["w1g"] = np.ascontiguousarray(
        w1.transpose(3, 2, 1, 0).reshape(27, 32).astype(np.float32))
    d["w2g"] = np.ascontiguousarray(
        w2.transpose(3, 2, 1, 0).reshape(3, 96, 64).astype(np.float32))
    w3t = w3.transpose(3, 2, 1, 0).reshape(3, 192, 64).astype(np.float32)
    d["w3a"] = np.ascontiguousarray(w3t[:, :128])
    d["w3b"] = np.ascontiguousarray(w3t[:, 128:])
    d["cb1"] = np.tile(b1.astype(np.float32), 4).reshape(128, 1)
    d["cb2"] = np.tile(b2.astype(np.float32), 2).reshape(128, 1)
    d["cb3"] = (0.4 * np.tile(b3.astype(np.float32), 2)).reshape(128, 1)
    # fc1: permute input features to f=(s, c) ordering; tiles [p, k, m, n]
    perm = np.array([c * 16 + s for s in range(16) for c in range(64)])
    fc1t = fc1_w.T[perm].astype(np.float32)            # [1024, 512]
    a = fc1t.reshape(8, 128, 4, 128).transpose(1, 0, 2, 3).reshape(128, -1)
    d["fc1h"], d["fc1l"] = split16(a)
    fc2t = fc2_w.T.astype(np.float32)                  # [512, 256]
    a = fc2t.reshape(4, 128, 2, 128).transpose(1, 0, 2, 3).reshape(128, -1)
    d["fc2h"], d["fc2l"] = split16(a)
    lit = li_w.T.astype(np.float32)                    # [256, 10]
    a = lit.reshape(2, 128, 10).transpose(1, 0, 2).reshape(128, 20)
    d["lih"], d["lil"] = split16(a)
    d["id10"] = np.eye(10, dtype=np.float32)
    assert not np.any(fc1_b) and not np.any(fc2_b), \
        "nonzero fc biases not implemented"
    return d


def kernel(x, w1, b1, w2, b2, w3, b3, fc1_w, fc1_b, fc2_w, fc2_b, li_w,
           trace=False):
    global LAST_EXEC_NS
    if "nc" not in _CACHE:
        _CACHE["nc"] = build_nc()
    nc = _CACHE["nc"]
    wd = prep_weights(w1, b1, w2, b2, w3, b3, fc1_w, fc1_b, fc2_w, fc2_b, li_w)
    in_maps = []
    for c in range(N_CORES):
        m = dict(wd)
        xs = x[c * BPC:(c + 1) * BPC].astype(np.float32)
        m["xp"] = np.pad(xs, ((0, 0), (0, 0), (1, 1), (1, 1)))
        in_maps.append(m)
    res = run_bass_kernel_spmd(nc, in_maps, list(range(N_CORES)), trace=trace)
    LAST_EXEC_NS = res.exec_time_ns
    return np.concatenate([res.results[c]["out"] for c in range(N_CORES)], 0)


# revision 20
# speedup vs baseline: 1.6864x; 1.6159x over previous
"""Trainium2 Bass kernel for nn_C3SNN_ModelT: CNN feature extractor + LIF SNN.

Data parallel over 8 cores (128 samples each). Per core:
  - conv stage: 3x (conv3x3 SAME + relu + maxpool2x2), fp32 matmuls.
    L1 uses a DRAM-staged im2col (K=27, single pass); L2/L3 use ky-replicated
    padded rows with kx handled by accumulating matmul passes. Col-tiled PSUM
    packing keeps relu/pool epilogues on all 128 partitions; pooling runs
    before relu (they commute) straight out of PSUM via reduce_max.
  - SNN stage: 32 timesteps, feature-major layout (features on partitions,
    batch in free dim). FC matmuls use fp16 split weights (w = hi + lo, both
    fp16); spike inputs are {0,1} hence exact in fp16; PSUM accumulates fp32.
    LIF state updates are fused scalar_tensor_tensor ops on DVE; spike
    extraction runs on GPSIMD.
"""
import sys
sys.path.insert(0, "/opt/trn_rl_repo")

import numpy as np
import concourse.bass as bass
import concourse.mybir as mybir
import concourse.tile as tile
from concourse import bacc
from concourse.bass_utils import run_bass_kernel_spmd

F32 = mybir.dt.float32
F16 = mybir.dt.float16
MAX = mybir.AluOpType.max
MULT = mybir.AluOpType.mult
ADD = mybir.AluOpType.add
IS_GT = mybir.AluOpType.is_gt
IS_LE = mybir.AluOpType.is_le
RELU = mybir.ActivationFunctionType.Relu
SIGN = mybir.ActivationFunctionType.Sign
AXX = mybir.AxisListType.X

N_CORES = 8
BPC = 128          # batch per core
BB = 8             # conv batch chunk
NCHUNK = BPC // BB
SEQ = 32

LAST_EXEC_NS = None
_CACHE = {}


def build_nc(debug_outputs=False, do_conv=True, seq=SEQ):
    nc = bacc.Bacc(None, target_bir_lowering=False, debug=False)

    # ---- DRAM I/O ----
    xp = nc.dram_tensor("xp", [BPC, 3, 34, 34], F32, kind="ExternalInput")
    w1g = nc.dram_tensor("w1g", [27, 32], F32, kind="ExternalInput")
    w2g = nc.dram_tensor("w2g", [3, 96, 64], F32, kind="ExternalInput")
    w3a = nc.dram_tensor("w3a", [3, 128, 64], F32, kind="ExternalInput")
    w3b = nc.dram_tensor("w3b", [3, 64, 64], F32, kind="ExternalInput")
    cb1 = nc.dram_tensor("cb1", [128, 1], F32, kind="ExternalInput")
    cb2 = nc.dram_tensor("cb2", [128, 1], F32, kind="ExternalInput")
    cb3 = nc.dram_tensor("cb3", [128, 1], F32, kind="ExternalInput")  # 0.4*b3
    fc1h = nc.dram_tensor("fc1h", [128, 8 * 4 * 128], F16, kind="ExternalInput")
    fc1l = nc.dram_tensor("fc1l", [128, 8 * 4 * 128], F16, kind="ExternalInput")
    fc2h = nc.dram_tensor("fc2h", [128, 4 * 2 * 128], F16, kind="ExternalInput")
    fc2l = nc.dram_tensor("fc2l", [128, 4 * 2 * 128], F16, kind="ExternalInput")
    lih = nc.dram_tensor("lih", [128, 2 * 10], F16, kind="ExternalInput")
    lil = nc.dram_tensor("lil", [128, 2 * 10], F16, kind="ExternalInput")
    id10 = nc.dram_tensor("id10", [10, 10], F32, kind="ExternalInput")
    out = nc.dram_tensor("out", [BPC, 10], F32, kind="ExternalOutput")
    dbg = {}
    if debug_outputs:
        dbg["featT"] = nc.dram_tensor("dbg_featT", [128, 8, 128], F32,
                                      kind="ExternalOutput")

    xr = xp[:].rearrange("b c h w -> c b (h w)")

    with tile.TileContext(nc) as tc:
        with (
            tc.tile_pool(name="wpool", bufs=1) as wpool,
            tc.tile_pool(name="state", bufs=1) as state,
        ):
            # weights to SBUF
            w1s = wpool.tile([27, 32], F32)
            w2s = wpool.tile([96, 3, 64], F32)
            w3as = wpool.tile([128, 3, 64], F32)
            w3bs = wpool.tile([64, 3, 64], F32)
            cb1s = wpool.tile([128, 1], F32)
            cb2s = wpool.tile([128, 1], F32)
            cb3s = wpool.tile([128, 1], F32)
            fc1hs = wpool.tile([128, 8 * 4 * 128], F16)
            fc1ls = wpool.tile([128, 8 * 4 * 128], F16)
            fc2hs = wpool.tile([128, 4 * 2 * 128], F16)
            fc2ls = wpool.tile([128, 4 * 2 * 128], F16)
            lihs = wpool.tile([128, 2 * 10], F16)
            lils = wpool.tile([128, 2 * 10], F16)
            id10s = wpool.tile([10, 10], F32)
            for dst_t, src_t in [(w1s, w1g), (cb1s, cb1), (cb2s, cb2),
                                 (cb3s, cb3), (fc1hs, fc1h), (fc1ls, fc1l),
                                 (fc2hs, fc2h), (fc2ls, fc2l), (lihs, lih),
                                 (lils, lil), (id10s, id10)]:
                nc.sync.dma_start(dst_t[:], src_t[:])
            for dst_t, src_t in [(w2s, w2g), (w3as, w3a), (w3bs, w3b)]:
                nc.sync.dma_start(dst_t[:],
                                  src_t[:].rearrange("k p n -> p k n"))

            # featT: scaled features (0.1*feat), f-layout [p=(sig,ch), t(8), b]
            featT = state.tile([128, 8, 128], F32)

            if do_conv:
                build_conv(nc, tc, xr, featT, w1s, w2s, w3as, w3bs,
                           cb1s, cb2s, cb3s)
            else:
                nc.vector.memset(featT[:], 0.0)

            if debug_outputs:
                nc.sync.dma_start(dbg["featT"][:], featT[:])

            build_snn(nc, tc, state, featT, fc1hs, fc1ls, fc2hs, fc2ls,
                      lihs, lils, id10s, out, seq)

    nc.compile()
    return nc


def build_conv(nc, tc, xr, featT, w1s, w2s, w3as, w3bs,
               cb1s, cb2s, cb3s):
    # L1 im2col staged in DRAM: row p=(kx,ky,ci) holds padded rows shifted by
    # (ky, kx): im27d[p, b, i*34+j'] = xpad[ci, b, i+ky, j'+kx] via contiguous
    # runs; run-wrap garbage lands only in never-read pad columns j' >= 32.
    # Staged per chunk inside the loop so the DRAM->DRAM copies pipeline with
    # compute; HWDGE FIFO order on the sync queue sequences D2D before the
    # chunk's D2S load.
    with (
        tc.tile_pool(name="conv_in", bufs=1) as conv_in,
        tc.tile_pool(name="conv_sc", bufs=2) as csc,
        tc.tile_pool(name="dstage", bufs=1, space="DRAM") as dstage,
        tc.tile_pool(name="pl1", bufs=2, space="PSUM") as pl1,
        tc.tile_pool(name="pl23", bufs=2, space="PSUM") as pl23,
    ):
        im27t = dstage.tile([27, BPC, 1088], F32)
        im27v = im27t[:]
        # layout tiles; padded borders memset once: per-chunk DMAs only write
        # real interiors, the boundary zeros persist across chunks
        t27s = [conv_in.tile([27, BB, 1088], F32, tag=f"t27_{i}",
                             name=f"t27_{i}") for i in range(2)]
        l2pad = conv_in.tile([32, BB, 18, 18], F32, tag="l2p", name="l2p")
        rep96 = conv_in.tile([96, BB, 16, 18], F32, tag="r96", name="r96")
        l3pad = conv_in.tile([64, BB, 10, 10], F32, tag="l3p", name="l3p")
        repa = conv_in.tile([128, BB, 8, 10], F32, tag="ra", name="ra")
        repb = conv_in.tile([64, BB, 8, 10], F32, tag="rb", name="rb")
        nc.vector.memset(l2pad[:], 0.0)
        nc.vector.memset(l3pad[:], 0.0)

        for ci in range(NCHUNK):
            b0 = ci * BB
            t27 = t27s[ci % 2]
            for kx in range(3):
                for ky in range(3):
                    p0 = 3 * (kx * 3 + ky)
                    s0 = ky * 34 + kx
                    L = min(1088, 1156 - s0)
                    nc.gpsimd.dma_start(im27v[p0:p0 + 3, b0:b0 + BB, 0:L],
                                        xr[0:3, b0:b0 + BB, s0:s0 + L])
                    if L < 1088:
                        # junk tail lands in never-read pad cols; keeps
                        # CoreSim's uninitialized-read checks quiet
                        with nc.allow_non_contiguous_dma(reason="pad tail"):
                            nc.gpsimd.dma_start(
                                im27v[p0:p0 + 3, b0:b0 + BB, L:1088],
                                xr[0:3, b0:b0 + BB, 0:1088 - L])
            # ---- L1: load staged im2col chunk; 4 rounds x 4 col-groups ----
            nc.sync.dma_start(t27[:], im27v[0:27, b0:b0 + BB, :])
            t27view = t27[:].rearrange("p b (i j) -> p b i j", j=34)
            for rnd in range(4):
                ps = pl1.tile([128, 512], F32, tag="ps1", name="ps1")
                for c in range(4):
                    u = rnd * 4 + c
                    smp, nh = u // 2, u % 2
                    nc.tensor.matmul(
                        ps[32 * c:32 * c + 32, :], w1s[:, :],
                        t27view[0:27, smp, 16 * nh:16 * nh + 16, 0:32],
                        start=True, stop=True, tile_position=(0, 32 * c))
                r4 = ps[:].rearrange("p (i j two) -> p i j two",
                                     i=16, j=16, two=2)
                p1t = csc.tile([128, 16, 16], F32, tag="cpa", name="cpa1")
                nc.vector.reduce_max(p1t[:], r4, axis=AXX)
                p14 = p1t[:].rearrange("p (i two) j -> p i two j", two=2)
                p2t = csc.tile([128, 8, 16], F32, tag="cpb", name="cpb1")
                nc.vector.tensor_tensor(p2t[:], p14[:, :, 0, :],
                                        p14[:, :, 1, :], MAX)
                p2r = csc.tile([128, 8, 16], F32, tag="cpr", name="cpr1")
                nc.scalar.activation(p2r[:], p2t[:], RELU, bias=cb1s[:])
                for c in range(4):
                    u = rnd * 4 + c
                    smp, nh = u // 2, u % 2
                    nc.sync.dma_start(
                        l2pad[0:32, smp, 1 + 8 * nh:9 + 8 * nh, 1:17],
                        p2r[32 * c:32 * c + 32, :, :])

            # ---- L2: ky-replicate + 3 kx passes, col-pack x2 ----
            for ky in range(3):
                nc.sync.dma_start(rep96[32 * ky:32 * ky + 32, :],
                                  l2pad[0:32, :, ky:ky + 16, :])
            for n2 in range(2):
                ps = pl23.tile([128, 512], F32, tag="ps2", name="ps2")
                for c in range(2):
                    for kx in range(3):
                        nc.tensor.matmul(
                            ps[64 * c:64 * c + 64, :], w2s[:, kx, :],
                            rep96[0:96, c * 4 + n2 * 2:c * 4 + n2 * 2 + 2,
                                  :, kx:kx + 16],
                            start=(kx == 0), stop=(kx == 2),
                            tile_position=(0, 64 * c))
                r4 = ps[:].rearrange("p (si j two) -> p si j two",
                                     si=32, j=8, two=2)
                p1t = csc.tile([128, 32, 8], F32, tag="cpa", name="cpa2")
                nc.vector.reduce_max(p1t[:], r4, axis=AXX)
                p14 = p1t[:].rearrange("p (a two) j -> p a two j", two=2)
                p2t = csc.tile([128, 2, 8, 8], F32, tag="cpb", name="cpb2")
                p2tv = p2t[:].rearrange("p s i j -> p (s i) j")
                nc.vector.tensor_tensor(p2tv, p14[:, :, 0, :],
                                        p14[:, :, 1, :], MAX)
                p2r = csc.tile([128, 2, 8, 8], F32, tag="cpr", name="cpr2")
                nc.scalar.activation(p2r[:], p2t[:], RELU, bias=cb2s[:])
                for c in range(2):
                    s0 = c * 4 + n2 * 2
                    for si in range(2):
                        nc.sync.dma_start(
                            l3pad[0:64, s0 + si, 1:9, 1:9],
                            p2r[64 * c:64 * c + 64, si, :, :])

            # ---- L3: ky-replicate + matmuls, col-pack x2 ----
            for ky in range(2):
                nc.sync.dma_start(repa[64 * ky:64 * ky + 64, :],
                                  l3pad[0:64, :, ky:ky + 8, :])
            nc.sync.dma_start(repb[0:64, :], l3pad[0:64, :, 2:10, :])
            ps3 = pl23.tile([128, 256], F32, tag="ps3", name="ps3")
            for c in range(2):
                for kx in range(3):
                    nc.tensor.matmul(
                        ps3[64 * c:64 * c + 64, :], w3as[:, kx, :],
                        repa[0:128, c * 4:c * 4 + 4, :, kx:kx + 8],
                        start=(kx == 0), stop=False,
                        tile_position=(0, 64 * c))
                    nc.tensor.matmul(
                        ps3[64 * c:64 * c + 64, :], w3bs[:, kx, :],
                        repb[0:64, c * 4:c * 4 + 4, :, kx:kx + 8],
                        start=False, stop=(kx == 2),
                        tile_position=(0, 64 * c))
            r4 = ps3[:].rearrange("p (si j two) -> p si j two",
                                  si=32, j=4, two=2)
            p1t = csc.tile([128, 32, 4], F32, tag="cpa", name="cpa3")
            nc.vector.reduce_max(p1t[:], r4, axis=AXX)
            p14 = p1t[:].rearrange("p (s i two) j -> p s i two j",
                                   s=4, i=4, two=2)
            # pass2 writes (q, s)-major flat layout: elem q*4 + s
            p2p = csc.tile([128, 64], F32, tag="cpb", name="cpb3")
            p2pv = p2p[:].rearrange("p (i j s) -> p s i j", i=4, j=4, s=4)
            nc.vector.tensor_tensor(p2pv, p14[:, :, :, 0, :],
                                    p14[:, :, :, 1, :], MAX)
            # relu(0.4*x + 0.4*b3) = 0.4*relu(x + b3); folds CNN_SCALER*DT_TM
            p2t = csc.tile([128, 64], F32, tag="cpr", name="cpr3")
            nc.scalar.activation(p2t[:], p2p[:], RELU, bias=cb3s[:], scale=0.4)
            # featT assembly: spatial q = i*4+j = 2t + sig; feature f = q*64+ch
            p2q = p2t[:].rearrange("p (t two s) -> p t two s", t=8, two=2, s=4)
            for sig in range(2):
                for c in range(2):
                    src = p2q[64 * c:64 * c + 64, :, sig, :]
                    dst = featT[64 * sig:64 * sig + 64, :,
                                b0 + 4 * c:b0 + 4 * c + 4]
                    if sig == c:
                        nc.vector.tensor_copy(dst.opt(), src.opt())
                    else:
                        nc.sync.dma_start(dst.opt(), src.opt())


def build_snn(nc, tc, state, featT, fc1hs, fc1ls, fc2hs, fc2ls, lihs, lils,
              id10s, out, seq):
    with (
        tc.tile_pool(name="snn_sc", bufs=1) as ssc,
        tc.tile_pool(name="pc1", bufs=2, space="PSUM") as pc1,
        tc.tile_pool(name="pc2", bufs=2, space="PSUM") as pc2,
        tc.tile_pool(name="pli", bufs=2, space="PSUM") as pli,
    ):
        ve = state.tile([128, 8, 128], F32)
        vs1 = state.tile([128, 4, 128], F32)   # 10*v1
        i1 = state.tile([128, 4, 128], F32)
        vs2 = state.tile([128, 2, 128], F32)   # 10*v2
        i2 = state.tile([128, 2, 128], F32)
        wl = state.tile([10, 128], F32)        # 10*vl
        il = state.tile([10, 128], F32)
        z16 = state.tile([128, 8, 128], F16)
        s116 = state.tile([128, 4, 128], F16)
        s216 = state.tile([128, 2, 128], F16)
        for t_ in (ve, vs1, i1, vs2, i2, wl, il):
            nc.vector.memset(t_[:], 0.0)

        fc1h4 = fc1hs.rearrange("p (k m n) -> p k m n", k=8, m=4)
        fc1l4 = fc1ls.rearrange("p (k m n) -> p k m n", k=8, m=4)
        fc2h4 = fc2hs.rearrange("p (k m n) -> p k m n", k=4, m=2)
        fc2l4 = fc2ls.rearrange("p (k m n) -> p k m n", k=4, m=2)
        lih4 = lihs.rearrange("p (k n) -> p k n", k=2)
        lil4 = lils.rearrange("p (k n) -> p k n", k=2)

        for t in range(seq):
            # encoder: ve = 0.9*ve + 0.1*feat; z = ve>1; ve *= (ve<=1)
            nc.vector.scalar_tensor_tensor(
                ve[:], ve[:], 0.9, featT[:], MULT, ADD)
            nc.vector.tensor_scalar(z16[:], ve[:], 1.0, None, IS_GT)
            nc.vector.scalar_tensor_tensor(
                ve[:], ve[:], 1.0, ve[:], IS_LE, MULT)

            # fc1: cur1 = fc1_w @ z  (f-layout out [512, 128])
            ps1 = pc1.tile([128, 4, 128], F32, tag="ps1", name="sps1")
            for m in range(4):
                for k in range(8):
                    nc.tensor.matmul(
                        ps1[:, m, :], fc1h4[:, k, m, :], z16[:, k, :],
                        start=(k == 0), stop=False)
                for k in range(8):
                    nc.tensor.matmul(
                        ps1[:, m, :], fc1l4[:, k, m, :], z16[:, k, :],
                        start=False, stop=(k == 7))

            # LIF1 (state scaled by 10; th=4.0): v1d uses OLD i1
            v1d = ssc.tile([128, 4, 128], F32, tag="scrA", name="v1d")
            nc.vector.scalar_tensor_tensor(
                v1d[:], vs1[:], 0.9, i1[:], MULT, ADD)
            nc.vector.tensor_scalar(s116[:], v1d[:], 4.0, None, IS_GT)
            nc.vector.scalar_tensor_tensor(
                vs1[:], v1d[:], 4.0, v1d[:], IS_LE, MULT)
            nc.vector.scalar_tensor_tensor(
                i1[:], i1[:], 0.8, ps1[:], MULT, ADD)

            # fc2
            ps2 = pc2.tile([128, 2, 128], F32, tag="ps2", name="sps2")
            for m in range(2):
                for k in range(4):
                    nc.tensor.matmul(
                        ps2[:, m, :], fc2h4[:, k, m, :], s116[:, k, :],
                        start=(k == 0), stop=False)
                for k in range(4):
                    nc.tensor.matmul(
                        ps2[:, m, :], fc2l4[:, k, m, :], s116[:, k, :],
                        start=False, stop=(k == 3))

            # LIF2
            v2d = ssc.tile([128, 2, 128], F32, tag="scrA", name="v2d")
            nc.vector.scalar_tensor_tensor(
                v2d[:], vs2[:], 0.9, i2[:], MULT, ADD)
            nc.vector.tensor_scalar(s216[:], v2d[:], 4.0, None, IS_GT)
            nc.vector.scalar_tensor_tensor(
                vs2[:], v2d[:], 4.0, v2d[:], IS_LE, MULT)
            nc.vector.scalar_tensor_tensor(
                i2[:], i2[:], 0.8, ps2[:], MULT, ADD)

            # LILinear: ij = il + li_w @ s2; wl = 0.9wl + ij; il = 0.8*ij
            psl = pli.tile([10, 128], F32, tag="psl", name="psl")
            nc.tensor.matmul(psl[:], lih4[:, 0, :], s216[:, 0, :],
                             start=True, stop=False)
            nc.tensor.matmul(psl[:], lih4[:, 1, :], s216[:, 1, :],
                             start=False, stop=False)
            nc.tensor.matmul(psl[:], lil4[:, 0, :], s216[:, 0, :],
                             start=False, stop=False)
            nc.tensor.matmul(psl[:], lil4[:, 1, :], s216[:, 1, :],
                             start=False, stop=True)
            ij = ssc.tile([10, 128], F32, tag="scrB", name="ij")
            nc.vector.tensor_tensor(ij[:], il[:], psl[:], ADD)
            nc.vector.scalar_tensor_tensor(
                wl[:], wl[:], 0.9, ij[:], MULT, ADD)
            nc.vector.tensor_scalar(il[:], ij[:], 0.8, None, MULT)

        # output: out[b, n] = 0.1 * wl[n, b] via PE transpose
        vlT = state.tile([10, 128], F32)
        nc.vector.tensor_scalar(vlT[:], wl[:], 0.1, None, MULT)
        with tc.tile_pool(name="pout", bufs=1, space="PSUM") as pout:
            pso = pout.tile([128, 10], F32)
            nc.tensor.transpose(pso[:], vlT[:], id10s[:])
            ot = state.tile([128, 10], F32)
            nc.vector.tensor_copy(ot[:], pso[:])
            nc.sync.dma_start(out[:], ot[:])


def prep_weights(w1, b1, w2, b2, w3, b3, fc1_w, fc1_b, fc2_w, fc2_b, li_w):
    def split16(a):
        hi = a.astype(np.float16)
        lo = (a - hi.astype(np.float32)).astype(np.float16)
        return hi, lo

    d = {}
    d["w1g"] = np.ascontiguousarray(
        w1.transpose(3, 2, 1, 0).reshape(27, 32).astype(np.float32))
    d["w2g"] = np.ascontiguousarray(
        w2.transpose(3, 2, 1, 0).reshape(3, 96, 64).astype(np.float32))
    w3t = w3.transpose(3, 2, 1, 0).reshape(3, 192, 64).astype(np.float32)
    d["w3a"] = np.ascontiguousarray(w3t[:, :128])
    d["w3b"] = np.ascontiguousarray(w3t[:, 128:])
    d["cb1"] = np.tile(b1.astype(np.float32), 4).reshape(128, 1)
    d["cb2"] = np.tile(b2.astype(np.float32), 2).reshape(128, 1)
    d["cb3"] = (0.4 * np.tile(b3.astype(np.float32), 2)).reshape(128, 1)
    # fc1: permute input features to f=(s, c) ordering; tiles [p, k, m, n]
    perm = np.array([c * 16 + s for s in range(16) for c in range(64)])
    fc1t = fc1_w.T[perm].astype(np.float32)            # [1024, 512]
    a = fc1t.reshape(8, 128, 4, 128).transpose(1, 0, 2, 3).reshape(128, -1)
    d["fc1h"], d["fc1l"] = split16(a)
    fc2t = fc2_w.T.astype(np.float32)                  # [512, 256]
    a = fc2t.reshape(4, 128, 2, 128).transpose(1, 0, 2, 3).reshape(128, -1)
    d["fc2h"], d["fc2l"] = split16(a)
    lit = li_w.T.astype(np.float32)                    # [256, 10]
    a = lit.reshape(2, 128, 10).transpose(1, 0, 2).reshape(128, 20)
    d["lih"], d["lil"] = split16(a)
    d["id10"] = np.eye(10, dtype=np.float32)
    assert not np.any(fc1_b) and not np.any(fc2_b), \
        "nonzero fc biases not implemented"
    return d


def kernel(x, w1, b1, w2, b2, w3, b3, fc1_w, fc1_b, fc2_w, fc2_b, li_w,
           trace=False):
    global LAST_EXEC_NS
    if "nc" not in _CACHE:
        _CACHE["nc"] = build_nc()
    nc = _CACHE["nc"]
    wd = prep_weights(w1, b1, w2, b2, w3, b3, fc1_w, fc1_b, fc2_w, fc2_b, li_w)
    in_maps = []
    for c in range(N_CORES):
        m = dict(wd)
        xs = x[c * BPC:(c + 1) * BPC].astype(np.float32)
        m["xp"] = np.pad(xs, ((0, 0), (0, 0), (1, 1), (1, 1)))
        in_maps.append(m)
    res = run_bass_kernel_spmd(nc, in_maps, list(range(N_CORES)), trace=trace)
    LAST_EXEC_NS = res.exec_time_ns
    return np.concatenate([res.results[c]["out"] for c in range(N_CORES)], 0)
